# revision 16
# baseline (speedup 1.0000x reference)
"""DCT-II enhancement kernel for Trainium2 (8 NeuronCores, data parallel).

Computes out[b, n, k] = sum_d x[b, n, d] * C[k, d] where C is the 256x256
orthonormal DCT-II basis — i.e. a [B*N, 256] @ [256, 256]^T GEMM.

Sharding: pure data parallel over the flattened token dim (B*N = 131072),
16384 tokens per core. The DCT basis (transposed, [d, k]) is replicated.

Precision: the correctness gate is rel_err < 2e-2; the orthonormal basis
preserves norms, so bf16 I/O contributes only ~3e-3 relative error while
halving HBM traffic (the fp32 version sat on the fp32 DMA roofline at
~103us; bf16 floor is ~51us/core at ~330 GB/s).

Key bottleneck learned from traces: a single DMA ring sustains only
~110-145 GB/s, and only three rings exist (HWDGE on sync/scalar, SWDGE
on gpsimd). Traffic is balanced ~5.5MB/ring: input on gpsimd + sync,
output on scalar with sync absorbing the late iterations (when its input
work is done), 4KB descriptors everywhere (1024-token iterations).

Layout: the host pre-transposes each core's shard to xT [d=256, 16384]
bf16, tokens permuted so the device's natural tiling (psum block s of
iteration I holds token I*1024 + p*8 + s at partition p) writes the
output in natural row-major order. Device = pure matmul pipeline, no PE
transposes:

  per 1024-token iteration:
    1. DMA in xT [128p(d), 2c, tok] bf16 (4KB runs, 2048-token granules,
       alternating sync/gpsimd rings).
    2. 16 matmuls into 4 PSUM banks: out[tok=128, k=256] += xTc.T @ CTc
       (lhsT = xT slice 128d x 128tok, rhs = CT chunk 128d x 256k).
    3. 4 PSUM->SBUF copies with fp32->bf16 cast (2 on DVE, 2 on ACT).
    4. DMA out [128p, 8, 256] bf16 (4KB contiguous per partition),
       alternating scalar/vector rings; last iterations ship each half
       as soon as its copies land to shorten the tail drain.
"""

from contextlib import ExitStack

import ml_dtypes
import numpy as np

import concourse.bass as bass
import concourse.tile as tile
from concourse import bacc, mybir
from concourse.bass_utils import run_bass_kernel_spmd

P = 128
D = 256
N_CORES = 8
B, N = 32, 4096
TOK_PER_CORE = (B * N) // N_CORES  # 16384

F32 = mybir.dt.float32
BF16 = mybir.dt.bfloat16
NP_BF16 = ml_dtypes.bfloat16


def dct_matrix() -> np.ndarray:
    """C[k, d] — DCT-II with ortho normalization, fp64 math cast to fp32."""
    n = D
    k = np.arange(n)[:, None].astype(np.float64)
    m = np.arange(n)[None, :].astype(np.float64)
    Cm = np.cos(np.pi * (2.0 * m + 1.0) * k / (2.0 * n))
    scale = np.full((n, 1), np.sqrt(2.0 / n))
    scale[0, 0] = np.sqrt(1.0 / n)
    return (Cm * scale).astype(np.float32)


def build_program(tok: int = TOK_PER_CORE, super_tok: int = 1024,
                  num_devices: int = N_CORES) -> bass.Bass:
    """Emit the per-core Bass/Tile program. All cores run the same NEFF."""
    assert tok % super_tok == 0 and super_tok % (2 * P) == 0
    nit = tok // super_tok       # 1024-token iterations (16)
    tb = super_tok // P          # tokens per output partition per iter (8)
    dc = D // P                  # contraction chunks (2 x 128)
    gr = 2 * super_tok           # input DMA granule: 2048 tokens, 4KB runs
    ngr = tok // gr

    nc = bacc.Bacc(
        "TRN2", target_bir_lowering=False, debug=False, num_devices=num_devices
    )
    xt_d = nc.dram_tensor("xt", [D, tok], BF16, kind="ExternalInput").ap()
    ct_d = nc.dram_tensor("ct", [D, D], BF16, kind="ExternalInput").ap()
    out_d = nc.dram_tensor("out", [tok, D], BF16, kind="ExternalOutput").ap()

    with ExitStack() as ctx:
        tc = ctx.enter_context(tile.TileContext(nc))
        consts = ctx.enter_context(tc.tile_pool(name="consts", bufs=1))
        xin_pool = ctx.enter_context(tc.tile_pool(name="xin", bufs=6))
        out_sb_pool = ctx.enter_context(tc.tile_pool(name="out_sb", bufs=4))
        # Each PSUM tile spans 2 banks ([128, 1024] fp32); 4 bufs = all
        # 8 banks, 2 iterations in flight.
        out_ps_pool = ctx.enter_context(
            tc.tile_pool(name="out_ps", bufs=4, space="PSUM")
        )

        # CT as [p, c, k] (d = c*128 + p), first on the scalar ring — the
        # very first matmul needs it. Two DMAs so the c=0 chunk lands in
        # half the time (matmuls are ordered c0-first).
        ct_sb = consts.tile([P, dc, D], BF16)
        ct_r = ct_d.rearrange("(c p) k -> p c k", p=P)
        nc.scalar.dma_start(ct_sb[:, 0:1, :], ct_r[:, 0:1, :])
        nc.scalar.dma_start(ct_sb[:, 1:2, :], ct_r[:, 1:2, :])

        # xT column j of iteration I holds token I*1024 + p*8 + s where
        # j = s*128 + p (host-side permutation) -> psum block s lands
        # tokens contiguously per partition for 4KB-run out DMAs.
        x_half = xt_d.rearrange("(c p) (h t) -> h p c t", p=P, t=super_tok // 2)
        x_fill = xt_d.rearrange("(c p) (i t) -> i p c t", p=P, t=super_tok)
        x_gr = xt_d.rearrange("(c p) (g t) -> g p c t", p=P, t=gr)
        o_t = out_d.rearrange("(i p s) k -> i p s k", p=P, s=tb)

        x_q = xt_d.rearrange("(c p) (q t) -> q p c t", p=P, t=super_tok // 4)

        xins = {}

        def stage_a_fill0():
            """Iteration 0 lands as 256/256/512-token tiles so the first
            matmuls start after only 128KB of input."""
            qa = xin_pool.tile([P, dc, super_tok // 4], BF16, name="xf0a")
            qb = xin_pool.tile([P, dc, super_tok // 4], BF16, name="xf0b")
            hc = xin_pool.tile([P, dc, super_tok // 2], BF16, name="xf0c")
            nc.sync.dma_start(qa[:], x_q[0])
            nc.sync.dma_start(qb[:], x_q[1])
            nc.sync.dma_start(hc[:], x_half[1])
            xins[0] = ("quads", qa, qb, hc)

        def stage_a_fill(i, eng):
            xc = xin_pool.tile([P, dc, super_tok], BF16, name=f"xfill{i}")
            eng.dma_start(xc[:], x_fill[i])
            xins[i] = ("one", xc, 0)

        # Granule ring schedule: sync takes g2 and g5 (its fill work ends
        # early), gpsimd the rest (its SWDGE queue measures ~196 GB/s vs
        # ~130-145 for the HWDGE rings); sync and gpsimd then absorb mid
        # and late out DMAs.
        GR_SYNC = {2, 5}

        def stage_a(g):
            """Granule g covers iterations 2g, 2g+1 (g >= 1)."""
            if not (1 <= g < ngr):
                return
            xg = xin_pool.tile([P, dc, gr], BF16)
            eng = nc.sync if g in GR_SYNC else nc.gpsimd
            eng.dma_start(xg[:], x_gr[g])
            xins[2 * g] = ("one", xg, 0)
            xins[2 * g + 1] = ("one", xg, super_tok)

        pss_by_iter = {}

        def stage_b(i):
            """16 matmuls -> 2 two-bank PSUM tiles (4 accumulation groups
            each), ordered c0-first so iteration 0 only gates on the c=0
            halves of ct and the fill."""
            if not (0 <= i < nit):
                return
            ent = xins.pop(i)
            if ent[0] == "quads":
                _, qa, qb, hc = ent
                q = tb // 4

                def wslice(c, s):
                    if s < q:
                        return qa[:, c, s * P:(s + 1) * P]
                    if s < 2 * q:
                        return qb[:, c, (s - q) * P:(s - q + 1) * P]
                    o = (s - 2 * q) * P
                    return hc[:, c, o:o + P]
            else:
                _, xg, off = ent

                def wslice(c, s):
                    o = off + s * P
                    return xg[:, c, o:o + P]

            pss = []
            for sp in range(2):
                ps = out_ps_pool.tile([P, (tb // 2) * D], F32)
                pss.append(ps)
                for s_in in range(tb // 2):
                    s = (tb // 2) * sp + s_in
                    for c in range(dc):
                        nc.tensor.matmul(
                            ps[:, s_in * D:(s_in + 1) * D],
                            wslice(c, s),
                            ct_sb[:, c, :],
                            start=(c == 0),
                            stop=(c == dc - 1),
                        )
            pss_by_iter[i] = pss

        # Out-DMA ring per iteration: scalar by default; sync mid-kernel
        # once its input granules are through; gpsimd (fastest queue,
        # input done by ~37us) takes the late iterations.
        OUT_RING = {5: nc.sync, 7: nc.sync, 9: nc.sync,
                    11: nc.gpsimd, 13: nc.gpsimd}

        def stage_c(i):
            """PSUM->SBUF bf16 copies (1 DVE + 1 ACT) + out DMA."""
            if not (0 <= i < nit):
                return
            pss = pss_by_iter.pop(i)
            out_sb = out_sb_pool.tile([P, tb, D], BF16)
            half = tb // 2
            if i >= nit - 2:
                # Tail drain: ship each half as soon as its copy lands,
                # on different rings (gpsimd + the other two, all idle
                # by now).
                ringa = nc.scalar if i % 2 == 0 else nc.gpsimd
                ringb = nc.gpsimd if i % 2 == 0 else nc.sync
                nc.vector.tensor_copy(out_sb[:, 0:half, :], pss[0][:])
                ringa.dma_start(o_t[i, :, 0:half, :], out_sb[:, 0:half, :])
                nc.scalar.copy(out_sb[:, half:tb, :], pss[1][:])
                ringb.dma_start(o_t[i, :, half:tb, :], out_sb[:, half:tb, :])
            else:
                ring = OUT_RING.get(i, nc.scalar)
                nc.vector.tensor_copy(out_sb[:, 0:half, :], pss[0][:])
                nc.scalar.copy(out_sb[:, half:tb, :], pss[1][:])
                ring.dma_start(o_t[i], out_sb[:])

        stage_a_fill0()
        stage_a_fill(1, nc.gpsimd)
        stage_a(1)
        for i in range(nit + 1):
            if i % 2 == 0:
                stage_a(i // 2 + 2)
            stage_b(i)
            stage_c(i - 1)

    nc.compile()
    return nc


_PROGRAM_CACHE: dict = {}


def _get_program() -> bass.Bass:
    if "nc" not in _PROGRAM_CACHE:
        _PROGRAM_CACHE["nc"] = build_program()
    return _PROGRAM_CACHE["nc"]


def make_in_maps(x_flat: np.ndarray) -> list[dict]:
    """x_flat: [B*N, D] float32. Cast to bf16 and pre-transpose each shard
    to xT [d, j] where column j = I*1024 + s*128 + p holds token
    I*1024 + p*8 + s (matches the device's psum-block tiling)."""
    ct = np.ascontiguousarray(dct_matrix().T).astype(NP_BF16)  # [d, k]
    xb = x_flat.astype(NP_BF16)
    nit = TOK_PER_CORE // 1024
    # [core, I, p, s, d] -> [core, d, I, s, p]
    xr = xb.reshape(N_CORES, nit, P, 8, D).transpose(0, 4, 1, 3, 2)
    xt = np.ascontiguousarray(xr).reshape(N_CORES, D, TOK_PER_CORE)
    return [{"xt": xt[i], "ct": ct} for i in range(N_CORES)]


def kernel(x: np.ndarray) -> np.ndarray:
    x = np.ascontiguousarray(np.asarray(x, dtype=np.float32))
    b, n, d = x.shape
    assert (b, n, d) == (B, N, D), f"unexpected shape {x.shape}"
    nc = _get_program()
    in_maps = make_in_maps(x.reshape(b * n, d))
    res = run_bass_kernel_spmd(nc, in_maps, core_ids=list(range(N_CORES)))
    out = np.concatenate(
        [np.asarray(r["out"]).astype(np.float32) for r in res.results], axis=0
    )
    return out.reshape(b, n, d)
